# revision 4
# baseline (speedup 1.0000x reference)
"""AttentionPool Trainium2 kernel.

Computes, for x [B, N, D], mask [B, N], q [D]:
    logits = einsum('bnd,d->bn', x, q);  logits[~mask] = -inf
    w = softmax(logits, axis=-1)
    out = einsum('bn,bnd->bd', w, x)

Sharding: data-parallel over B across 8 NeuronCores (4 rows per core).

Position enumeration (per row): n = p*64 + u, with p = SBUF partition
and u = column in [0,64): each partition owns 64 consecutive positions
= one contiguous 64 KiB DRAM run per (partition, row), so every DMA
piece is a single large descriptor per partition (fewer descriptors =
less SWDGE ring pressure). The (p,u) <-> n mapping cancels between
pass 1 and pass 2, so the softmax math is unaffected by the layout.

Design (memory-roofline targeted; trace-measured notes inline):
  - x is DMA'd via the SWDGE (gpsimd) path with an inline f32 -> fp16
    cast: HBM reads stay f32 (33.5 MB/core, the roofline; measured
    ~400-420 GB/s read-side on this part), SBUF holds fp16. This removes
    the ScalarE cast pass (~81 us busy in the f32+ScalarE-cast variant).
    fp16 (not bf16) because logit precision from 16-bit inputs is the
    dominant error term: bf16 inputs measured rel_err 2.0e-2, right at
    the 2e-2 gate; fp16 measures 1.9e-3.
  - Masked positions are zeroed in x ON THE HOST: their logit becomes
    exactly 0, so they contribute exp(-shift) to Z (subtracted on the
    host, which knows the mask count) and nothing to the weighted sum.
    No device-side mask/bias tensor at all.
  - The softmax shift is a host-side constant 4.5*||q||, applied as the
    [P, 1] bias of the exp activation. Any shift cancels in the host
    division by Z; it only must keep exp() in f32 range (row max is
    within [2.5, 4.6]*||q|| whp for randn inputs). This removes the
    GPSIMD partition_all_reduce and any cross-chunk barrier.
    w = exp(logits) stays bf16: its exponent range matches f32, while
    fp16 w would flush to zero for plausible shifts.
  - Logits via custom DVE scans (cumsum of x*q; stride-0 output AP keeps
    each 256-element segment end; segment dots = adjacent difference of
    ends, one [P, k] subtract per piece). Rows are processed in column
    PIECES (see ROW_PIECES): a small first piece starts the DVE early,
    mid pieces amortize the ~400-cycle scan op overhead, and the last
    row tapers to 2-col pieces so the post-stream tail is a ~0.7us scan
    plus the last exp/matmul, not a full-row drain. Scan throughput
    ~1.05-1.09 cyc/elem keeps DVE busy (~72us) under the DMA stream
    (~80us); DVE is the second-closest engine to critical.
  - Per piece: subtract (DVE), one exp with bias=-shift and accum_out
    partial-Z (ScalarE), and a burst of k/2 back-to-back matmuls
    (TensorE, M=2: lhsT = two w columns [128, 2] bf16, rhs = their two
    fp16 x tiles [128, 512] (mixed 16-bit operands are legal), one PSUM
    [2, 512] accumulation chain per row). Sustained MM bursts let the
    PE HAM clock reach 2.4 GHz.
  - Host combines the PSUM halves, corrects Z, and divides.

Known hazard: SDMA engine 15 occasionally runs ~15% slow (SWDGE
descriptor-ring port contention, stochastic), adding 10-20us on
afflicted runs; typical runs are ~104-115us, afflicted ~117-126us.
"""

import numpy as np

B, N, D = 32, 8192, 256
N_CORES = 8
B_LOC = B // N_CORES  # 4
P = 128
S = 8               # consecutive positions per partition (8 KiB descriptors)
T8 = N // (P * S)   # 8 t8 groups per row
T = N // P          # 64 tiles (columns) per row

# per-row DMA/scan piece sizes in COLUMNS (1 col = one (t8, s) position =
# 256 elements = 1 KiB/partition in DRAM): small first piece to start
# compute early, and a fine taper on the last row so the post-stream tail
# is a 2-col scan (~0.7us), not a multi-group one.
ROW_PIECES = (
    (4, 28, 32),
    (32, 32),
    (32, 32),
    (16, 16, 8, 8, 8, 4, 2, 2),
)
NPIECE = sum(len(p) for p in ROW_PIECES)

_cache = {}

_SCAN_OP_NAME = "ATTNPOOL_MUL_SCAN"


def _register_scan_op():
    """Register a custom DVE op computing scan(add, Src0*Src1) in-process.

    The stock TENSOR_TENSOR_REDUCE / TENSOR_TENSOR_SCAN opcodes crash this
    terminal's ucode; custom-DVE ops ship their own uop tables inside the
    NEFF, so they are self-contained.
    """
    from concourse import dve_ops
    from concourse.dve_spec import AluOp, Spec, Src0, Src1, scan, lower, _has_src1
    from concourse.dve_uop import DveOpSpec

    for op in dve_ops.OPS:
        if op.name == _SCAN_OP_NAME:
            return op
    spec = Spec(
        body=scan(AluOp.ADD, Src0 * Src1),
        reference=lambda in0, in1, c0, c1, c2: np.cumsum(
            in0.astype(np.float32) * in1.astype(np.float32), axis=1, dtype=np.float32
        ),
    )
    row = dve_ops._CUSTOM_DVE_ROW_BASE + len(dve_ops.OPS)
    assert row < 0x20
    shas = {}
    for ver in ("v3", "v4"):
        tmp = DveOpSpec(
            name=_SCAN_OP_NAME,
            opcode=row,
            uops=lower(spec, ver=ver),
            rd1_en=_has_src1(spec),
        )
        shas[ver] = tmp.sha(ver)
    op = dve_ops.DveOp(_SCAN_OP_NAME, spec, subdim=False, uops_sha=shas)
    dve_ops.OPS.append(op)
    dve_ops._SUB_OPCODE_FOR_NAME[_SCAN_OP_NAME] = row
    dve_ops.CUSTOM_DVE_SPECS[_SCAN_OP_NAME] = spec
    return op


def _build():
    import concourse.bass as bass
    import concourse.tile as tile
    from concourse import bacc, mybir, bass_isa

    scan_op = _register_scan_op()

    dt = mybir.dt
    nc = bacc.Bacc(
        "TRN2", target_bir_lowering=False, debug=False, num_devices=N_CORES
    )
    x_d = nc.dram_tensor("x", [B_LOC, N, D], dt.float16, kind="ExternalInput").ap()
    nshift_d = nc.dram_tensor(
        "nshift", [P, 1], dt.float32, kind="ExternalInput"
    ).ap()
    q_d = nc.dram_tensor("q", [P, D], dt.float16, kind="ExternalInput").ap()
    out_d = nc.dram_tensor(
        "out", [B_LOC, 2, 2 * D], dt.float32, kind="ExternalOutput"
    ).ap()
    z_d = nc.dram_tensor("z", [P, NPIECE], dt.float32, kind="ExternalOutput").ap()

    GE = T + max(len(p) for p in ROW_PIECES)  # ends cols: zero col per piece

    with tile.TileContext(nc) as tc:
        with (
            tc.tile_pool(name="singles", bufs=1) as singles,
            tc.tile_pool(name="xrow", bufs=4) as xrow_pool,
            tc.tile_pool(name="small", bufs=4) as small,
            tc.tile_pool(name="psum", bufs=4, space="PSUM") as psum,
        ):
            qb = singles.tile([P, D], dt.float16)
            nc.sync.dma_start(qb[:], q_d[:])
            nst = singles.tile([P, 1], dt.float32)
            nc.sync.dma_start(nst[:], nshift_d[:])
            zt = singles.tile([P, NPIECE], dt.float32)
            # persistent per-row ends tiles: rows have different piece
            # layouts, so each needs its own zero-column positions. The
            # zero columns are written once here and never touched again
            # (scans only write the segment-end columns via the stride-0
            # output AP), so no per-row memset is needed.
            ends_row = [
                singles.tile([P, GE], dt.float32, name=f"ends{j}")
                for j in range(B_LOC)
            ]
            for e in ends_row:
                nc.vector.memset(e[:], 0.0)

            zcol = 0
            for b in range(B_LOC):
                pieces = ROW_PIECES[b]
                assert sum(pieces) == T
                # position mapping n = p*T + u: partition p owns T=64
                # consecutive positions = one contiguous 64 KiB DRAM run per
                # (partition, row). Each piece is then ONE descriptor per
                # partition (k KiB), ~4-8x fewer descriptors than the old
                # (t8, p, s) interleave -> far less SWDGE descriptor-ring
                # traffic (the eng-7/15 + DVE-port contention hazard).
                xrow = x_d[b].rearrange("(p u) d -> p u d", p=P)
                rt = xrow_pool.tile([P, T, D], dt.float16)
                off = 0
                for k in pieces:
                    nc.sync.dma_start(rt[:, off : off + k], xrow[:, off : off + k])
                    off += k

                ends = ends_row[b]
                logits = small.tile([P, T], dt.float32)
                w = small.tile([P, T], dt.bfloat16)
                acc = psum.tile([2, 2 * D], dt.float32)

                col0 = 0  # column offset
                ecol = 0  # ends column: zero col at ecol, ends at ecol+1..
                for k in pieces:
                    o3 = (
                        ends[:, ecol + 1 : ecol + 1 + k]
                        .rearrange("p (k u) -> p k u", u=1)
                        .broadcast_to([P, k, D])
                    )
                    nc.vector._custom_dve(
                        scan_op,
                        out=o3,
                        in0=rt[:, col0 : col0 + k],
                        in1=qb.rearrange("p (u d) -> p u d", u=1).broadcast_to(
                            [P, k, D]
                        ),
                    )
                    nc.vector.tensor_tensor(
                        logits[:, col0 : col0 + k],
                        ends[:, ecol + 1 : ecol + 1 + k],
                        ends[:, ecol : ecol + k],
                        op=mybir.AluOpType.subtract,
                    )
                    nc.scalar.activation(
                        w[:, col0 : col0 + k],
                        logits[:, col0 : col0 + k],
                        mybir.ActivationFunctionType.Exp,
                        bias=nst[:],
                        accum_out=zt[:, zcol : zcol + 1],
                    )
                    for col in range(col0, col0 + k, 2):
                        nc.tensor.matmul(
                            acc[:],
                            w[:, col : col + 2],
                            rt[:, col : col + 2].rearrange("p s d -> p (s d)"),
                            start=(col == 0),
                            stop=(col == T - 2),
                        )
                    col0 += k
                    ecol += k + 1
                    zcol += 1

                halves = small.tile([2, 2 * D], dt.float32)
                nc.scalar.copy(halves[:], acc[:])
                nc.sync.dma_start(out_d[b], halves[:])
            nc.scalar.dma_start(z_d[:], zt[:])

    nc.compile()
    return nc


def _prep_core_inputs(x, mask, q):
    """Host-side shard prep. Returns (per-core input dicts, shift).

    Masked positions are zeroed in x itself: their logit becomes exactly 0,
    so they contribute exp(-shift) to Z (subtracted on the host) and
    nothing to the weighted sum (w * 0). This removes the device-side bias
    tensor and its per-piece add entirely.
    """
    qb = np.ascontiguousarray(
        np.broadcast_to(q[None, :], (P, D))
    ).astype(np.float16)
    shift = np.float32(4.5 * np.linalg.norm(q.astype(np.float64)))
    nshift = np.full((P, 1), -shift, dtype=np.float32)
    in_maps = []
    for i in range(N_CORES):
        sl = slice(i * B_LOC, (i + 1) * B_LOC)
        # fp16 cast on the host: the device then reads 16.8 MB/core instead
        # of 33.5 MB (the inline SWDGE f32->fp16 cast kept HBM reads f32).
        # Numerics are identical to the inline-cast variant (same fp16 x).
        xm = (x[sl] * mask[sl][:, :, None]).astype(np.float16)
        in_maps.append(
            {
                "x": np.ascontiguousarray(xm),
                "nshift": nshift,
                "q": qb,
            }
        )
    return in_maps, shift


def kernel(x, mask, q, _trace=False):
    from concourse.bass_utils import run_bass_kernel_spmd

    x = np.asarray(x, dtype=np.float32)
    mask = np.asarray(mask)
    q = np.asarray(q, dtype=np.float32)
    assert x.shape == (B, N, D) and mask.shape == (B, N) and q.shape == (D,)

    if "nc" not in _cache:
        _cache["nc"] = _build()
    nc = _cache["nc"]

    in_maps, shift = _prep_core_inputs(x, mask, q)
    res = run_bass_kernel_spmd(nc, in_maps, list(range(N_CORES)), trace=_trace)

    # piece -> row mapping for the partial-Z columns
    row_of_piece = []
    for b, pieces in enumerate(ROW_PIECES):
        row_of_piece += [b] * len(pieces)
    row_of_piece = np.array(row_of_piece)

    # each masked position contributed exp(0 - shift) to Z
    emshift = np.exp(np.float64(-shift))
    n_masked = (~mask).sum(axis=1).astype(np.float64)  # [B]

    out = np.empty((B, D), dtype=np.float32)
    for i in range(N_CORES):
        sl = slice(i * B_LOC, (i + 1) * B_LOC)
        h = res.results[i]["out"]  # [B_LOC, 2, 512] PSUM halves, unnormalized
        o = h[:, 0, 0:D] + h[:, 1, D : 2 * D]
        zp = res.results[i]["z"].astype(np.float64)  # [P, NPIECE]
        z = np.array(
            [zp[:, row_of_piece == b].sum() for b in range(B_LOC)]
        )
        z -= n_masked[sl] * emshift
        out[sl] = o / z[:, None]
    if _trace:
        return out, res
    return out



# revision 17
# speedup vs baseline: 1.1123x; 1.1123x over previous
"""AttentionPool Trainium2 kernel.

Computes, for x [B, N, D], mask [B, N], q [D]:
    logits = einsum('bnd,d->bn', x, q);  logits[~mask] = -inf
    w = softmax(logits, axis=-1)
    out = einsum('bn,bnd->bd', w, x)

Sharding: data-parallel over B across 8 NeuronCores (4 rows per core).

Position enumeration (per row): n = p*64 + u, with p = SBUF partition
and u = column in [0,64): each partition owns 64 consecutive positions
= one contiguous DRAM run per (partition, row), so every DMA piece is a
single large descriptor per partition.

Design (memory-roofline targeted; trace-measured notes inline):
  - x is masked (zeroed) and cast to fp16 ON THE HOST: the device reads
    16.8 MB/core instead of 33.5 MB f32 (measured ~420 GB/s/core -> the
    x stream is ~40 us). Plain HWDGE (nc.sync) loads - no SWDGE cast
    pass, no gpsimd descriptor-ring hazard. fp16 (not bf16) because
    logit precision from 16-bit inputs is the dominant error term: the
    softmax here is extremely peaked (||q|| ~ 16), so bf16 logits move
    top-2 weight splits by ~2% (rel_err 2.0e-2, at the gate); fp16
    measures 1.9e-3.
  - Masked positions are zeroed in x: their logit becomes exactly 0,
    contributing exp(-shift) to Z (subtracted on the host) and nothing
    to the weighted sum. No device-side mask tensor.
  - The softmax shift is a host-side constant 4.5*||q||, applied as the
    [P, 1] bias of the exp activation (any shift cancels in the host
    division by Z; it only must keep exp() in f32 range).
  - Pass 1 (logits) is SPLIT between DVE and TensorE. The DVE runs a
    custom scan (cumsum of x*q, stride-0 output AP keeps segment ends;
    segment dots = adjacent difference) at its hard cap of 1 elem/cyc
    (HW-probed: custom DVE ops run mode=Disable; the SRC_0_HI crossbar
    lane reads 0.0 outside true 2x mode, so a packed pair-scan is not
    possible). Full pass 1 on DVE = ~70 us > everything else, so the
    LAST GCOLS columns of each row are computed on the TensorE instead:
    the host uploads those columns PRE-TRANSPOSED (xT[b, dlo, c, u, p])
    and the PE does, per (row, c-chunk, u): LDWEIGHTS(xT tile [dlo, p])
    + matmul(rhs=q chunk [dlo, 1]) accumulating the two c-chunks into a
    PSUM logits column [p, 1] - directly in the [p, u] layout that exp
    and pass 2 need. Costs +GCOLS/64 x DMA but removes the same DVE
    fraction; LDW+MM(N=1) is NX-dispatch-bound (~220-320 ns/column).
  - exp on ScalarE: per DVE piece from SBUF, per row for the PE region
    from PSUM (ScalarE sits next to PSUM); both emit w in bf16 (fp16 w
    would flush to zero for plausible shifts; bf16 keeps f32 exponent
    range).
  - Pass 2 on TensorE as before: per 2 columns, lhsT = two w columns
    [128, 2] bf16, rhs = their x tiles [128, 512] fp16, one PSUM [2,
    512] accumulation chain per row. Pieces are kept at ~12 cols so PE
    bursts recur every ~3 us and the HAM clock gate stays at 8/8.
  - Z per row via ones^T @ w matmul (out [1, 64] PSUM, host sums the 64
    values): removes the per-piece accum_out + ACTIVATION_READ_
    ACCUMULATOR ops and the end-of-kernel z DMA; the Z columns are
    packed into the same `halves` output DMA.
"""

import numpy as np

B, N, D = 32, 8192, 256
N_CORES = 8
B_LOC = B // N_CORES  # 4
P = 128
T = N // P          # 64 tiles (columns) per row
GCOLS = 16          # trailing columns per row computed on the TensorE
DVE_T = T - GCOLS   # leading columns per row computed on the DVE

# per-row DVE piece sizes in COLUMNS (1 col = 128 positions x 256 d).
# Small first pieces start the DVE early; ~12-col steady pieces keep the
# pass-2 matmul bursts ~3 us apart (PE HAM stays warm); the last row
# tapers so the post-stream tail is short.
ROW_PIECES = (
    (2, 6, 8, 16, 16),
    (12, 12, 12, 12),
    (12, 12, 12, 12),
    (12, 12, 8, 8, 4, 4),
)
ZCOLS = 64  # Z columns packed after the 2*D output halves

_cache = {}

_SCAN_OP_NAME = "ATTNPOOL_MUL_SCAN"


def _register_scan_op():
    """Register a custom DVE op computing scan(add, Src0*Src1) in-process.

    The stock TENSOR_TENSOR_REDUCE / TENSOR_TENSOR_SCAN opcodes crash this
    terminal's ucode; custom-DVE ops ship their own uop tables inside the
    NEFF, so they are self-contained.
    """
    from concourse import dve_ops
    from concourse.dve_spec import AluOp, Spec, Src0, Src1, scan, lower, _has_src1
    from concourse.dve_uop import DveOpSpec

    for op in dve_ops.OPS:
        if op.name == _SCAN_OP_NAME:
            return op
    spec = Spec(
        body=scan(AluOp.ADD, Src0 * Src1),
        reference=lambda in0, in1, c0, c1, c2: np.cumsum(
            in0.astype(np.float32) * in1.astype(np.float32), axis=1, dtype=np.float32
        ),
    )
    row = dve_ops._CUSTOM_DVE_ROW_BASE + len(dve_ops.OPS)
    assert row < 0x20
    shas = {}
    for ver in ("v3", "v4"):
        tmp = DveOpSpec(
            name=_SCAN_OP_NAME,
            opcode=row,
            uops=lower(spec, ver=ver),
            rd1_en=_has_src1(spec),
        )
        shas[ver] = tmp.sha(ver)
    op = dve_ops.DveOp(_SCAN_OP_NAME, spec, subdim=False, uops_sha=shas)
    dve_ops.OPS.append(op)
    dve_ops._SUB_OPCODE_FOR_NAME[_SCAN_OP_NAME] = row
    dve_ops.CUSTOM_DVE_SPECS[_SCAN_OP_NAME] = spec
    return op


def _build():
    import concourse.bass as bass
    import concourse.tile as tile
    from concourse import bacc, mybir, bass_isa

    scan_op = _register_scan_op()

    dt = mybir.dt
    nc = bacc.Bacc(
        "TRN2", target_bir_lowering=False, debug=False, num_devices=N_CORES
    )
    x_d = nc.dram_tensor("x", [B_LOC, N, D], dt.float16, kind="ExternalInput").ap()
    xT_d = nc.dram_tensor(
        "xT", [B_LOC, P, 2, GCOLS, P], dt.float16, kind="ExternalInput"
    ).ap()
    nshift_d = nc.dram_tensor(
        "nshift", [P, 1], dt.float32, kind="ExternalInput"
    ).ap()
    q_d = nc.dram_tensor("q", [P, D], dt.float16, kind="ExternalInput").ap()
    qT_d = nc.dram_tensor("qT", [P, 2], dt.float16, kind="ExternalInput").ap()
    ones_d = nc.dram_tensor("ones", [P, 1], dt.float16, kind="ExternalInput").ap()
    out_d = nc.dram_tensor(
        "out", [B_LOC, 2, 2 * D + ZCOLS], dt.float32, kind="ExternalOutput"
    ).ap()

    GE = DVE_T + max(len(p) for p in ROW_PIECES)  # ends cols: zero col per piece

    with tile.TileContext(nc) as tc:
        with (
            tc.tile_pool(name="singles", bufs=1) as singles,
            tc.tile_pool(name="xrow", bufs=1) as xrow_pool,
            tc.tile_pool(name="xtt", bufs=1) as xtt_pool,
            tc.tile_pool(name="small", bufs=4) as small,
            tc.tile_pool(name="psum", bufs=2, space="PSUM") as psum,
            tc.tile_pool(name="psum1", bufs=2, space="PSUM") as psum1,
        ):
            qb = singles.tile([P, D], dt.float16)
            nc.sync.dma_start(qb[:], q_d[:])

            # row-0 first piece: issued as early as possible so the first
            # scan is DVE-preamble-bound, not DMA-bound.
            xrow0 = x_d[0].rearrange("(p u) d -> p u d", p=P)
            rt0 = xrow_pool.tile([P, T, D], dt.float16, name="rt0")
            k0 = ROW_PIECES[0][0]
            nc.sync.dma_start(rt0[:, 0:k0], xrow0[:, 0:k0])

            nst = singles.tile([P, 1], dt.float32)
            nc.sync.dma_start(nst[:], nshift_d[:])
            qT = singles.tile([P, 2], dt.float16)
            nc.sync.dma_start(qT[:], qT_d[:])
            ones = singles.tile([P, 1], dt.float16)
            nc.sync.dma_start(ones[:], ones_d[:])

            # persistent per-row ends tiles: zero columns written once; the
            # scans only write the segment-end columns (stride-0 output AP).
            ends_row = [
                singles.tile([P, GE], dt.float32, name=f"ends{j}")
                for j in range(B_LOC)
            ]
            nc.vector.memset(ends_row[0][:], 0.0)  # rows 1-3 memset later

            rts = [rt0]
            xtts = []
            for b in range(B_LOC):
                xrow = x_d[b].rearrange("(p u) d -> p u d", p=P)
                if b > 0:
                    rt = xrow_pool.tile([P, T, D], dt.float16, name=f"rt{b}")
                    rts.append(rt)
                rt = rts[b]
                start_idx = 1 if b == 0 else 0  # row-0 piece 0 issued above
                off = sum(ROW_PIECES[b][:start_idx])
                for k in ROW_PIECES[b][start_idx:]:
                    nc.sync.dma_start(rt[:, off : off + k], xrow[:, off : off + k])
                    off += k
                # PE-region columns of the normal layout (pass 2 needs them)
                nc.sync.dma_start(rt[:, DVE_T:T], xrow[:, DVE_T:T])
                # transposed copy for pass 1 on the PE
                xtt = xtt_pool.tile([P, 2, GCOLS, P], dt.float16, name=f"xtt{b}")
                nc.sync.dma_start(xtt[:], xT_d[b])
                xtts.append(xtt)

            for b in range(B_LOC):
                pieces = ROW_PIECES[b]
                assert sum(pieces) == DVE_T
                rt, xtt, ends = rts[b], xtts[b], ends_row[b]
                if b > 0:
                    # off the startup path: runs at the row boundary
                    nc.vector.memset(ends[:], 0.0)
                logits = small.tile([P, DVE_T], dt.float32)
                w = small.tile([P, T], dt.bfloat16)
                acc = psum.tile([2, 2 * D], dt.float32)
                # one PSUM bank: logits columns [P, GCOLS] + Z row [1, ZCOLS]
                lgz = psum1.tile([P, GCOLS + ZCOLS], dt.float32)
                lg = lgz[:, 0:GCOLS]
                zp = lgz[0:1, GCOLS : GCOLS + ZCOLS]

                # interleave plan: after each piece's pass-2 burst, issue a
                # slice of the row's pass-1 PE columns (data-ready early) so
                # the PE fills the DVE-paced gaps and the HAM stays warm.
                np_pieces = len(pieces)
                g_per = [GCOLS // np_pieces] * np_pieces
                for i in range(GCOLS - sum(g_per)):
                    g_per[i] += 1

                col0 = 0
                ecol = 0
                gj = 0
                for pi, k in enumerate(pieces):
                    o3 = (
                        ends[:, ecol + 1 : ecol + 1 + k]
                        .rearrange("p (k u) -> p k u", u=1)
                        .broadcast_to([P, k, D])
                    )
                    nc.vector._custom_dve(
                        scan_op,
                        out=o3,
                        in0=rt[:, col0 : col0 + k],
                        in1=qb.rearrange("p (u d) -> p u d", u=1).broadcast_to(
                            [P, k, D]
                        ),
                    )
                    nc.vector.tensor_tensor(
                        logits[:, col0 : col0 + k],
                        ends[:, ecol + 1 : ecol + 1 + k],
                        ends[:, ecol : ecol + k],
                        op=mybir.AluOpType.subtract,
                    )
                    nc.scalar.activation(
                        w[:, col0 : col0 + k],
                        logits[:, col0 : col0 + k],
                        mybir.ActivationFunctionType.Exp,
                        bias=nst[:],
                    )
                    for col in range(col0, col0 + k, 2):
                        nc.tensor.matmul(
                            acc[:],
                            w[:, col : col + 2],
                            rt[:, col : col + 2].rearrange("p s d -> p (s d)"),
                            start=(col == 0),
                            stop=(col == T - 2),
                        )
                    # pass-1 PE columns for this row, a slice per piece
                    for j in range(gj, gj + g_per[pi]):
                        nc.tensor.matmul(
                            lg[:, j : j + 1],
                            xtt[:, 0, j],
                            qT[:, 0:1],
                            start=True,
                            stop=False,
                        )
                        nc.tensor.matmul(
                            lg[:, j : j + 1],
                            xtt[:, 1, j],
                            qT[:, 1:2],
                            start=False,
                            stop=True,
                        )
                    gj += g_per[pi]
                    col0 += k
                    ecol += k + 1

                # PE-region exp (PSUM -> SBUF) and its pass-2 matmuls
                nc.scalar.activation(
                    w[:, DVE_T:T],
                    lg,
                    mybir.ActivationFunctionType.Exp,
                    bias=nst[:],
                )
                for col in range(DVE_T, T, 2):
                    nc.tensor.matmul(
                        acc[:],
                        w[:, col : col + 2],
                        rt[:, col : col + 2].rearrange("p s d -> p (s d)"),
                        start=(col == 0),
                        stop=(col == T - 2),
                    )
                # Z row = ones^T @ w -> [1, 64]; host sums the 64 values
                nc.tensor.matmul(zp, ones[:], w[:], start=True, stop=True)

                halves = small.tile([2, 2 * D + ZCOLS], dt.float32)
                nc.scalar.copy(halves[:, : 2 * D], acc[:])
                nc.scalar.copy(halves[0:1, 2 * D :], zp)
                # ACT HWDGE ring: does not queue behind the x stream (FIFO
                # per physical ring), and issues right after the copies
                nc.scalar.dma_start(out_d[b], halves[:])

    nc.compile()
    return nc


def _prep_core_inputs(x, mask, q):
    """Host-side shard prep. Returns (per-core input dicts, shift)."""
    qb = np.ascontiguousarray(
        np.broadcast_to(q[None, :], (P, D))
    ).astype(np.float16)
    qT = np.ascontiguousarray(q.reshape(2, P).T).astype(np.float16)  # [dlo, c]
    ones = np.ones((P, 1), dtype=np.float16)
    shift = np.float32(4.5 * np.linalg.norm(q.astype(np.float64)))
    nshift = np.full((P, 1), -shift, dtype=np.float32)
    in_maps = []
    for i in range(N_CORES):
        sl = slice(i * B_LOC, (i + 1) * B_LOC)
        # fp16 cast on the host: the device then reads 16.8 MB/core instead
        # of 33.5 MB (the inline SWDGE f32->fp16 cast kept HBM reads f32).
        xm = (x[sl] * mask[sl][:, :, None]).astype(np.float16)
        # pre-transposed trailing GCOLS columns: [b, p, u, c, dlo] ->
        # [b, dlo, c, u, p] so each partition (dlo) reads one contiguous run
        xv = xm.reshape(B_LOC, P, T, 2, P)
        xT = np.ascontiguousarray(xv[:, :, DVE_T:].transpose(0, 4, 3, 2, 1))
        in_maps.append(
            {
                "x": np.ascontiguousarray(xm),
                "xT": xT,
                "nshift": nshift,
                "q": qb,
                "qT": qT,
                "ones": ones,
            }
        )
    return in_maps, shift


def kernel(x, mask, q, _trace=False):
    from concourse.bass_utils import run_bass_kernel_spmd

    x = np.asarray(x, dtype=np.float32)
    mask = np.asarray(mask)
    q = np.asarray(q, dtype=np.float32)
    assert x.shape == (B, N, D) and mask.shape == (B, N) and q.shape == (D,)

    if "nc" not in _cache:
        _cache["nc"] = _build()
    nc = _cache["nc"]

    in_maps, shift = _prep_core_inputs(x, mask, q)
    res = run_bass_kernel_spmd(nc, in_maps, list(range(N_CORES)), trace=_trace)

    # each masked position contributed exp(0 - shift) to Z
    emshift = np.exp(np.float64(-shift))
    n_masked = (~mask).sum(axis=1).astype(np.float64)  # [B]

    out = np.empty((B, D), dtype=np.float32)
    for i in range(N_CORES):
        sl = slice(i * B_LOC, (i + 1) * B_LOC)
        h = res.results[i]["out"]  # [B_LOC, 2, 512+ZCOLS]
        o = h[:, 0, 0:D] + h[:, 1, D : 2 * D]
        z = h[:, 0, 2 * D :].astype(np.float64).sum(axis=1)
        z -= n_masked[sl] * emshift
        out[sl] = o / z[:, None]
    if _trace:
        return out, res
    return out
